# revision 1
# baseline (speedup 1.0000x reference)
"""Causal single-head attention on 8 TRN2 NeuronCores.

Problem: x:(S=4096, B=4, E=5) f32; Wk/Wq/Wv:(5,64), bk/bq/bv:(64,).
  K/Q/V = x@W + b per batch; scores = K·Q^T/8 (keys i, queries j), causal
  (key i attends query j iff i <= j), softmax over keys per query, out =
  sum_i V[i]*P[i,j] -> (S, B, 64).

Sharding: 8 cores = 4 batches x 2 query-stripe sets. Core parity 0 takes
query tiles at offsets {0,1024,2048,3072}, parity 1 {512,1536,2560,3584}.
All cores run one SPMD graph with a static per-slot i-block profile
fcnt=(4,12,20,28); per-core differences are pure input data:
  - "full" units use natural-order key blocks, with slot-specific x6 copies
    whose rows beyond the slot's causal prefix are zeroed (so slack blocks
    contribute nothing to numerator or denominator),
  - "diagonal" units use the core's own query rows as keys, so the 4
    triangular masks are core-invariant compile-time constants
    (built on-device with affine_select).

Math: biases via augmented 6-row inputs (ones row); softmax denominator via
an implicit ones-column: O^T = wv6^T @ (X6 @ P) where X6 @ P accumulates in
PSUM over key blocks; column 64 of wv6 selects the ones row -> denominator.
No max-subtraction (scores are O(1), exp is safe in f32).
"""

import sys
from contextlib import ExitStack

import ml_dtypes
import numpy as np

for _p in ("/opt/trn_rl_repo", "/opt/pypackages"):
    if _p not in sys.path:
        sys.path.append(_p)

import concourse.bass as bass
import concourse.tile as tile
from concourse import bacc, mybir

F32 = mybir.dt.float32
F32R = mybir.dt.float32r
BF16 = mybir.dt.bfloat16

S, B, E, NE = 4096, 4, 5, 64
N_CORES = 8
JT = 512          # query tile width
NSLOT = 4         # query tiles per core
FCNT = (4, 12, 20, 28)          # static full-unit (key-block) count per slot
F_OFF = (0, 4, 16, 36)          # cumulative offsets into x6full blocks
NFULL = sum(FCNT)               # 64 blocks
NDIAG = NSLOT * 4               # 16 blocks
JOS_BY_PARITY = ((0, 1024, 2048, 3072), (512, 1536, 2560, 3584))

_NC_CACHE = {}


def build_graph():
    nc = bacc.Bacc("TRN2", target_bir_lowering=False, debug=False)

    xt6k = nc.declare_dram_parameter("xt6k", [6, S], BF16, isOutput=False)
    xt6q = nc.declare_dram_parameter("xt6q", [6, NSLOT * JT], BF16, isOutput=False)
    x6full = nc.declare_dram_parameter("x6full", [128, NFULL * 6], BF16, isOutput=False)
    x6d = nc.declare_dram_parameter("x6d", [128, NDIAG * 6], BF16, isOutput=False)
    wk6 = nc.declare_dram_parameter("wk6", [6, 128], BF16, isOutput=False)
    wq6 = nc.declare_dram_parameter("wq6", [6, 128], BF16, isOutput=False)
    wv6pad = nc.declare_dram_parameter("wv6pad", [38, NE + 1], F32R, isOutput=False)
    out = nc.declare_dram_parameter("out", [NSLOT * JT, NE], F32, isOutput=True)

    with tile.TileContext(nc) as tc, ExitStack() as ctx:
        consts = ctx.enter_context(tc.tile_pool(name="consts", bufs=1))
        psum = ctx.enter_context(tc.tile_pool(name="psum", bufs=2, space="PSUM"))
        sb = ctx.enter_context(tc.tile_pool(name="sb", bufs=2))

        # ---- load constants / inputs into SBUF (critical path first) ----
        xt6k_sb = consts.tile([6, S], BF16)
        xt6q_sb = consts.tile([6, NSLOT * JT], BF16)
        x6full_sb = consts.tile([128, NFULL, 6], BF16)
        x6d_sb = consts.tile([128, NDIAG, 6], BF16)
        wk6_sb = consts.tile([6, 128], BF16)
        wq6_sb = consts.tile([6, 128], BF16)
        wv6pad_sb = consts.tile([38, NE + 1], F32R)
        nc.sync.dma_start(out=wk6_sb[:], in_=wk6[:])
        nc.sync.dma_start(out=wq6_sb[:], in_=wq6[:])
        nc.sync.dma_start(out=xt6q_sb[:], in_=xt6q[:])
        nc.sync.dma_start(out=xt6k_sb[:], in_=xt6k[:])

        # PE warmup: ~4.5us of dummy matmuls while the bulk DMAs land, so the
        # HAM clock gate reaches 8/8 before the real work starts.
        warm_ps = psum.tile([128, JT], F32, tag="xp", bufs=1)
        for _ in range(10):
            nc.tensor.matmul(
                warm_ps[:, :], wk6_sb[:], xt6q_sb[:, 0:JT], start=True, stop=True
            )

        nc.sync.dma_start(out=wv6pad_sb[:], in_=wv6pad[:])
        nc.sync.dma_start(
            out=x6full_sb[:], in_=x6full[:].rearrange("p (n c) -> p n c", c=6)
        )
        nc.sync.dma_start(out=x6d_sb[:], in_=x6d[:].rearrange("p (n c) -> p n c", c=6))

        # ---- projections: KT2 (replicated rows 0-63 / 64-127), QT2, KTd2 ----
        kt2 = consts.tile([128, S], BF16)
        qt2 = consts.tile([128, NSLOT * JT], BF16)
        ktd2 = consts.tile([128, NDIAG * 128], BF16)

        def project_pair(dst, w_sb, rhs_ap, col0, ncols):
            ps = psum.tile([128, ncols], F32, tag="st", bufs=2)
            nc.tensor.matmul(ps[:, :], w_sb[:], rhs_ap, start=True, stop=True)
            nc.vector.tensor_copy(dst[:, col0 : col0 + ncols], ps[:])

        for t4 in range(NSLOT):
            project_pair(qt2, wq6_sb, xt6q_sb[:, t4 * JT : (t4 + 1) * JT], t4 * JT, JT)
        for u8 in range(S // JT):
            project_pair(kt2, wk6_sb, xt6k_sb[:, u8 * JT : (u8 + 1) * JT], u8 * JT, JT)
        for t4 in range(NSLOT):
            project_pair(ktd2, wk6_sb, xt6q_sb[:, t4 * JT : (t4 + 1) * JT], t4 * JT, JT)

        # identity for PE transpose (memset/affine_select need f32; copy to f32r)
        ident_f = consts.tile([128, 128], F32)
        from concourse.masks import make_identity

        make_identity(nc, ident_f[:])

        # diagonal causal masks, core-invariant: mask_d[p, q] = (p + 128d <= q)
        masks_f = consts.tile([128, 4 * JT], F32)
        nc.gpsimd.memset(masks_f[:], 1.0)
        for d in range(4):
            nc.gpsimd.affine_select(
                out=masks_f[:, d * JT : (d + 1) * JT],
                in_=masks_f[:, d * JT : (d + 1) * JT],
                compare_op=mybir.AluOpType.is_ge,
                fill=0.0,
                base=-128 * d,
                pattern=[[1, JT]],
                channel_multiplier=-1,
            )
        masks_sb = consts.tile([128, 4 * JT], BF16)
        nc.vector.tensor_copy(masks_sb[:], masks_f[:])


        # ---- main attention loop (biggest slot first: shorter tail) ----
        for t in (3, 2, 1, 0):
            jcol = t * JT
            xp_ps = psum.tile([128, JT], F32, tag="xp", bufs=1)
            nf = FCNT[t]

            def unit_pair(lhs_src, ue_col, uo_col, xe, xo, start, stop, mask_pair):
                """One row-packed mm1 pair + exp (+ mask) + col-packed mm2a pair."""
                st_ps = psum.tile([128, 2 * JT], F32, tag="st", bufs=2)
                nc.tensor.matmul(
                    st_ps[:, 0:JT],
                    lhs_src[0:64, ue_col : ue_col + 128],
                    qt2[0:64, jcol : jcol + JT],
                    start=True,
                    stop=True,
                )
                nc.tensor.matmul(
                    st_ps[:, JT : 2 * JT],
                    lhs_src[64:128, uo_col : uo_col + 128],
                    qt2[64:128, jcol : jcol + JT],
                    start=True,
                    stop=True,
                )
                pt = sb.tile([128, 2 * JT], BF16, tag="pt", bufs=3)
                nc.scalar.activation(
                    pt[:], st_ps[:], mybir.ActivationFunctionType.Exp, scale=0.125
                )
                if mask_pair is not None:
                    nc.vector.tensor_mul(pt[:], pt[:], mask_pair)
                nc.tensor.matmul(
                    xp_ps[0:6, :], xe, pt[:, 0:JT], start=start, stop=stop,
                    skip_group_check=True,
                )
                nc.tensor.matmul(
                    xp_ps[32:38, :], xo, pt[:, JT : 2 * JT], start=start, stop=stop,
                    skip_group_check=True,
                )

            for up in range(nf // 2):
                ue, uo = 2 * up, 2 * up + 1
                unit_pair(
                    kt2,
                    ue * 128,
                    uo * 128,
                    x6full_sb[:, F_OFF[t] + ue, :],
                    x6full_sb[:, F_OFF[t] + uo, :],
                    start=(up == 0),
                    stop=False,
                    mask_pair=None,
                )
            for dp in range(2):
                de, do = 2 * dp, 2 * dp + 1
                unit_pair(
                    ktd2,
                    (4 * t + de) * 128,
                    (4 * t + do) * 128,
                    x6d_sb[:, 4 * t + de, :],
                    x6d_sb[:, 4 * t + do, :],
                    start=False,
                    stop=(dp == 1),
                    mask_pair=masks_sb[:, 2 * dp * JT : 2 * (dp + 1) * JT],
                )

            # ---- epilogue: O^T = wv6^T @ XP, then divide + transpose + out ----
            xp_sb = sb.tile([38, JT], F32R, tag="xps", bufs=2)
            nc.vector.tensor_copy(xp_sb[0:6, :], xp_ps[0:6, :])
            nc.vector.tensor_copy(xp_sb[32:38, :], xp_ps[32:38, :])
            ot_ps = psum.tile([NE + 1, JT], F32, tag="ot", bufs=1)
            nc.tensor.matmul(
                ot_ps[:], wv6pad_sb[0:6, :], xp_sb[0:6, :], start=True, stop=False
            )
            nc.tensor.matmul(
                ot_ps[:], wv6pad_sb[32:38, :], xp_sb[32:38, :], start=False, stop=True
            )
            ot_sb = sb.tile([NE + 1, JT], F32, tag="ots", bufs=2)
            nc.vector.tensor_copy(ot_sb[:], ot_ps[:])
            for s in range(JT // 128):
                tr_ps = psum.tile([128, NE + 1], F32, tag="tr", bufs=2)
                nc.tensor.transpose(
                    tr_ps[:, :],
                    ot_sb[:, s * 128 : (s + 1) * 128],
                    ident_f[0 : NE + 1, 0 : NE + 1],
                )
                rec = sb.tile([128, 1], F32, tag="rec", bufs=2)
                nc.vector.reciprocal(rec[:], tr_ps[:, NE : NE + 1])
                o_sb = sb.tile([128, NE], F32, tag="o", bufs=2)
                nc.vector.tensor_scalar_mul(o_sb[:], tr_ps[:, 0:NE], rec[:])
                r0 = t * JT + s * 128
                nc.sync.dma_start(out=out[r0 : r0 + 128, :], in_=o_sb[:])

    nc.compile()
    return nc


def make_in_maps(x, Wk, bk, Wq, bq, Wv, bv):
    """Build the 8 per-core input dicts from the full problem inputs."""
    x = np.asarray(x, np.float32)
    wk6_1 = np.vstack([np.asarray(Wk, np.float32), np.asarray(bk, np.float32)[None, :]])
    wq6_1 = np.vstack([np.asarray(Wq, np.float32), np.asarray(bq, np.float32)[None, :]])
    wk6 = np.hstack([wk6_1, wk6_1]).astype(ml_dtypes.bfloat16)  # both psum halves
    wq6 = np.hstack([wq6_1, wq6_1]).astype(ml_dtypes.bfloat16)
    wv6 = np.zeros((6, NE + 1), np.float32)
    wv6[0:5, 0:NE] = np.asarray(Wv, np.float32)
    wv6[5, 0:NE] = np.asarray(bv, np.float32)
    wv6[5, NE] = 1.0
    wv6pad = np.zeros((38, NE + 1), np.float32)
    wv6pad[0:6] = wv6
    wv6pad[32:38] = wv6

    in_maps = []
    for core in range(N_CORES):
        b, parity = core // 2, core % 2
        jos = JOS_BY_PARITY[parity]
        x6 = np.concatenate([x[:, b, :], np.ones((S, 1), np.float32)], axis=1)  # (S,6)

        xt6k = np.ascontiguousarray(x6.T).astype(ml_dtypes.bfloat16)  # (6, S)
        xt6q = np.ascontiguousarray(
            np.concatenate([x6[jo : jo + JT].T for jo in jos], axis=1)
        ).astype(ml_dtypes.bfloat16)  # (6, 2048)

        x6full = np.zeros((128, NFULL, 6), np.float32)
        for t, jo in enumerate(jos):
            blk = x6[: FCNT[t] * 128].copy().reshape(FCNT[t], 128, 6)
            nreal = jo // 128  # real causal-prefix blocks for this slot
            blk[nreal:] = 0.0
            x6full[:, F_OFF[t] : F_OFF[t] + FCNT[t], :] = blk.transpose(1, 0, 2)

        x6d = np.zeros((128, NDIAG, 6), np.float32)
        for t, jo in enumerate(jos):
            blk = x6[jo : jo + JT].reshape(4, 128, 6)
            x6d[:, 4 * t : 4 * t + 4, :] = blk.transpose(1, 0, 2)

        in_maps.append(
            {
                "xt6k": xt6k,
                "xt6q": xt6q,
                "x6full": np.ascontiguousarray(
                    x6full.reshape(128, NFULL * 6)
                ).astype(ml_dtypes.bfloat16),
                "x6d": np.ascontiguousarray(x6d.reshape(128, NDIAG * 6)).astype(
                    ml_dtypes.bfloat16
                ),
                "wk6": wk6,
                "wq6": wq6,
                "wv6pad": wv6pad,
            }
        )
    return in_maps


def assemble_output(results):
    """Stitch 8 per-core (2048, 64) outputs into (S, B, NE)."""
    out = np.zeros((S, B, NE), np.float32)
    for core in range(N_CORES):
        b, parity = core // 2, core % 2
        jos = JOS_BY_PARITY[parity]
        co = results[core]["out"]
        for t, jo in enumerate(jos):
            out[jo : jo + JT, b, :] = co[t * JT : (t + 1) * JT, :]
    return out


def run_on_device(in_maps, trace=False):
    from concourse.bass_utils import run_bass_kernel_spmd

    if "nc" not in _NC_CACHE:
        _NC_CACHE["nc"] = build_graph()
    nc = _NC_CACHE["nc"]
    return run_bass_kernel_spmd(
        nc, in_maps, core_ids=list(range(N_CORES)), trace=trace
    )


def kernel(x, Wk, bk, Wq, bq, Wv, bv):
    in_maps = make_in_maps(x, Wk, bk, Wq, bq, Wv, bv)
    res = run_on_device(in_maps, trace=False)
    return assemble_output(res.results)



# revision 4
# speedup vs baseline: 1.4165x; 1.4165x over previous
"""Causal single-head attention on 8 TRN2 NeuronCores (v2).

Problem: x:(S=4096, B=4, E=5) f32; Wk/Wq/Wv:(5,64), bk/bq/bv:(64,).
  K/Q/V = x@W + b per batch; scores = K.Q^T/8 (keys i, queries j), causal
  (key i attends query j iff i <= j), softmax over keys per query, out =
  sum_i V[i]*P[i,j] -> (S, B, 64).

Sharding: 8 cores = 4 batches x 2 query-stripe parities. Parity 0 takes
query tiles at offsets {0,1024,2048,3072}, parity 1 {512,1536,2560,3584}.
One SPMD graph; per-core differences are pure input data.

Key algebraic tricks (all host-precomputed):
  - scores = X6 @ M6 @ X6^T where X6 = [x | 1] (S,6) and M6 (6,6) folds
    Wk, Wq, both biases and the 1/sqrt(64) scale. G = X6 @ M6 is computed
    on host, so mm1 per 128-key block is a K=6 contraction:
    lhsT = G^T block (6,128), rhs = X6^T queries (6,512).
  - V6 = [x@Wv + bv | 1] (S,65); mm2 accumulates O^T = sum_blocks
    V6_blk^T @ P_blk directly into one PSUM bank per query slot. Column
    64 (the ones column) accumulates the softmax denominator.

Schedule per core: 4 query slots x 512; per slot, key blocks (128 keys)
grouped into "triad" units of <=3 blocks. Per unit: 3 row-tiled mm1
matmuls (tile groups at partitions 0/32/64 run concurrently), one Exp
activation over the whole [128, 3*512] PSUM unit (amortizes ACT
instruction overhead), DVE causal-mask multiplies on diagonal blocks,
then 3 accumulating mm2 matmuls. Slot epilogue: PE transpose + DVE
reciprocal/mul + DMA out. PSUM: st 2x3 banks, ot 1, tr 1 = 8.
No max-subtraction (scores are O(1), exp is safe).
"""

import sys
from contextlib import ExitStack

import ml_dtypes
import numpy as np

for _p in ("/opt/trn_rl_repo", "/opt/pypackages"):
    if _p not in sys.path:
        sys.path.append(_p)

import concourse.bass as bass
import concourse.tile as tile
from concourse import bacc, mybir

F32 = mybir.dt.float32
BF16 = mybir.dt.bfloat16

S, B, E, NE = 4096, 4, 5, 64
N_CORES = 8
JT = 512            # query tile width
NSLOT = 4
FCNT = (4, 12, 20, 28)   # static full-block count per slot (parity max)
JOS_BY_PARITY = ((0, 1024, 2048, 3072), (512, 1536, 2560, 3584))
UNIT_CAP = 3        # key blocks per unit (3 PSUM banks per st buffer)
SLOT_ORDER = (3, 2, 1, 0)   # biggest first: shorter tail

# ---- static unit tables (parity-independent structure) ----
# entry = ('F', f_idx) or ('D', d); per slot: FCNT full blocks + 4 diag.
SLOT_UNITS = []
for _t in range(NSLOT):
    _L = [("F", g) for g in range(FCNT[_t])] + [("D", d) for d in range(4)]
    SLOT_UNITS.append([_L[i : i + UNIT_CAP] for i in range(0, len(_L), UNIT_CAP)])
N_UNITS = sum(len(u) for u in SLOT_UNITS)            # 28
N_BLOCKS = sum(len(un) for u in SLOT_UNITS for un in u)  # 80
# flat offsets: UNIT_OFS[t][u] = unit index into g4 columns;
# BLK_OFS[t][u] = index of unit's first block into x6v blocks.
UNIT_OFS, BLK_OFS = [], []
_uc, _bc = 0, 0
for _t in range(NSLOT):
    _uo, _bo = [], []
    for _un in SLOT_UNITS[_t]:
        _uo.append(_uc)
        _bo.append(_bc)
        _uc += 1
        _bc += len(_un)
    UNIT_OFS.append(_uo)
    BLK_OFS.append(_bo)

_NC_CACHE = {}


def build_graph():
    nc = bacc.Bacc("TRN2", target_bir_lowering=False, debug=False)

    g4 = nc.declare_dram_parameter("g4", [128, N_UNITS * 128], BF16, isOutput=False)
    xq4 = nc.declare_dram_parameter("xq4", [128, NSLOT * JT], BF16, isOutput=False)
    x6v = nc.declare_dram_parameter("x6v", [128, N_BLOCKS * (NE + 1)], BF16,
                                    isOutput=False)
    out = nc.declare_dram_parameter("out", [NSLOT * JT, NE], F32, isOutput=True)

    with tile.TileContext(nc) as tc, ExitStack() as ctx:
        consts = ctx.enter_context(tc.tile_pool(name="consts", bufs=1))
        psum = ctx.enter_context(tc.tile_pool(name="psum", bufs=2, space="PSUM"))
        sb = ctx.enter_context(tc.tile_pool(name="sb", bufs=2))

        xq4_sb = consts.tile([128, NSLOT * JT], BF16)
        g4_sb = consts.tile([128, N_UNITS * 128], BF16)
        x6v_sb = consts.tile([128, N_BLOCKS * (NE + 1)], BF16)
        nc.sync.dma_start(out=xq4_sb[:], in_=xq4[:])
        nc.sync.dma_start(out=g4_sb[:], in_=g4[:])

        # PE warmup: dummy matmuls while bulk DMAs land so the HAM clock
        # gate reaches 8/8 before the real work starts.
        warm_ps = psum.tile([128, UNIT_CAP * JT], F32, tag="st", bufs=2)
        for _ in range(10):
            nc.tensor.matmul(
                warm_ps[:, 0:JT], xq4_sb[0:6, 0:128], xq4_sb[0:6, 0:JT],
                start=True, stop=True,
            )

        nc.sync.dma_start(out=x6v_sb[:], in_=x6v[:])

        # identity for PE transpose
        ident_f = consts.tile([128, 128], F32)
        from concourse.masks import make_identity

        make_identity(nc, ident_f[:])

        # diagonal causal masks, core-invariant: mask_d[p, q] = (p + 128d <= q)
        masks_f = consts.tile([128, 4 * JT], F32)
        nc.gpsimd.memset(masks_f[:], 1.0)
        for d in range(4):
            nc.gpsimd.affine_select(
                out=masks_f[:, d * JT : (d + 1) * JT],
                in_=masks_f[:, d * JT : (d + 1) * JT],
                compare_op=mybir.AluOpType.is_ge,
                fill=0.0,
                base=-128 * d,
                pattern=[[1, JT]],
                channel_multiplier=-1,
            )
        masks_sb = consts.tile([128, 4 * JT], BF16)
        nc.vector.tensor_copy(masks_sb[:], masks_f[:])

        # ---- main pipeline ----
        # front(k): mm1 unit k + Exp + masks.  back(k): mm2 accumulate
        # (+ slot epilogue when k closes a slot), emitted after front(k+1)
        # so the PE never waits on ACT(k) before starting mm1(k+1).
        fronts = []
        for t in SLOT_ORDER:
            for u, unit in enumerate(SLOT_UNITS[t]):
                fronts.append((t, u, unit))

        ot_tiles = {}

        def emit_front(t, u, unit):
            size = len(unit)
            st = psum.tile([128, UNIT_CAP * JT], F32, tag="st", bufs=2)
            gcol = UNIT_OFS[t][u] * 128
            for p in range(size):
                nc.tensor.matmul(
                    st[:, p * JT : (p + 1) * JT],
                    g4_sb[32 * p : 32 * p + 6, gcol : gcol + 128],
                    xq4_sb[32 * p : 32 * p + 6, t * JT : (t + 1) * JT],
                    start=True,
                    stop=True,
                )
            pt = sb.tile([128, UNIT_CAP * JT], BF16, tag="pt", bufs=3)
            nc.scalar.activation(
                pt[:, 0 : size * JT], st[:, 0 : size * JT],
                mybir.ActivationFunctionType.Exp,
            )
            for p, ent in enumerate(unit):
                if ent[0] == "D":
                    d = ent[1]
                    nc.vector.tensor_mul(
                        pt[:, p * JT : (p + 1) * JT],
                        pt[:, p * JT : (p + 1) * JT],
                        masks_sb[:, d * JT : (d + 1) * JT],
                    )
            return pt

        def emit_back(t, u, unit, pt):
            if u == 0:
                ot_tiles[t] = psum.tile(
                    [NE + 1, JT], F32, tag="ot", bufs=1, name="ot_ps"
                )
            ot_ps = ot_tiles[t]
            nu = len(SLOT_UNITS[t])
            for p in range(len(unit)):
                bi = BLK_OFS[t][u] + p
                first = u == 0 and p == 0
                last = u == nu - 1 and p == len(unit) - 1
                nc.tensor.matmul(
                    ot_ps[:, :],
                    x6v_sb[:, bi_col(t, bi) : bi_col(t, bi) + NE + 1],
                    pt[:, p * JT : (p + 1) * JT],
                    start=first,
                    stop=last,
                    skip_group_check=True,
                )
            if u == nu - 1:
                emit_epilogue(t, ot_ps)

        def bi_col(t, bi):
            # x6v blocks are laid out flat in slot-natural order; bi is the
            # per-slot block index; BLK_OFS already absolute, so bi is the
            # absolute block index here.
            return bi * (NE + 1)

        def emit_epilogue(t, ot_ps):
            ot_sb = sb.tile([NE + 1, JT], F32, tag="ots", bufs=2)
            nc.vector.tensor_copy(ot_sb[:], ot_ps[:])
            for s in range(JT // 128):
                tr_ps = psum.tile([128, NE + 1], F32, tag="tr", bufs=1)
                nc.tensor.transpose(
                    tr_ps[:, :],
                    ot_sb[:, s * 128 : (s + 1) * 128],
                    ident_f[0 : NE + 1, 0 : NE + 1],
                )
                rec = sb.tile([128, 1], F32, tag="rec", bufs=2)
                nc.vector.reciprocal(rec[:], tr_ps[:, NE : NE + 1])
                o_sb = sb.tile([128, NE], F32, tag="o", bufs=2)
                nc.vector.tensor_scalar_mul(o_sb[:], tr_ps[:, 0:NE], rec[:])
                r0 = t * JT + s * 128
                nc.sync.dma_start(out=out[r0 : r0 + 128, :], in_=o_sb[:])

        prev = None
        for t, u, unit in fronts:
            pt = emit_front(t, u, unit)
            if prev is not None:
                emit_back(*prev)
            prev = (t, u, unit, pt)
        emit_back(*prev)

    nc.compile()
    return nc


def make_in_maps(x, Wk, bk, Wq, bq, Wv, bv):
    """Build the 8 per-core input dicts from the full problem inputs."""
    x = np.asarray(x, np.float32)
    Wk = np.asarray(Wk, np.float32)
    bk = np.asarray(bk, np.float32)
    Wq = np.asarray(Wq, np.float32)
    bq = np.asarray(bq, np.float32)
    Wv = np.asarray(Wv, np.float32)
    bv = np.asarray(bv, np.float32)

    # M6 folds Wk/Wq/biases and the 1/sqrt(64) score scale.
    M6 = np.zeros((6, 6), np.float32)
    M6[0:5, 0:5] = Wk @ Wq.T
    M6[0:5, 5] = Wk @ bq
    M6[5, 0:5] = Wq @ bk
    M6[5, 5] = bk @ bq
    M6 *= 0.125

    in_maps = []
    per_batch = {}
    for b in range(B):
        X6 = np.concatenate([x[:, b, :], np.ones((S, 1), np.float32)], axis=1)
        G = X6 @ M6                                   # (S, 6)
        V6 = np.concatenate(
            [x[:, b, :] @ Wv + bv[None, :], np.ones((S, 1), np.float32)], axis=1
        )                                             # (S, 65)
        per_batch[b] = (X6, G, V6)

    for core in range(N_CORES):
        b, parity = core // 2, core % 2
        jos = JOS_BY_PARITY[parity]
        X6, G, V6 = per_batch[b]

        xq4 = np.zeros((128, NSLOT * JT), np.float32)
        for t in range(NSLOT):
            for grp in range(4):
                xq4[32 * grp : 32 * grp + 6, t * JT : (t + 1) * JT] = X6[
                    jos[t] : jos[t] + JT
                ].T
        g4 = np.zeros((128, N_UNITS * 128), np.float32)
        x6v = np.zeros((128, N_BLOCKS * (NE + 1)), np.float32)
        for t in range(NSLOT):
            jo = jos[t]
            nreal = jo // 128
            for u, unit in enumerate(SLOT_UNITS[t]):
                for p, ent in enumerate(unit):
                    if ent[0] == "F":
                        gb = ent[1]
                        slack = gb >= nreal
                    else:
                        gb = jo // 128 + ent[1]
                        slack = False
                    ucol = UNIT_OFS[t][u] * 128
                    g4[32 * p : 32 * p + 6, ucol : ucol + 128] = G[
                        128 * gb : 128 * gb + 128
                    ].T
                    bi = BLK_OFS[t][u] + p
                    if not slack:
                        x6v[:, bi * (NE + 1) : (bi + 1) * (NE + 1)] = V6[
                            128 * gb : 128 * gb + 128
                        ]

        in_maps.append(
            {
                "g4": np.ascontiguousarray(g4).astype(ml_dtypes.bfloat16),
                "xq4": np.ascontiguousarray(xq4).astype(ml_dtypes.bfloat16),
                "x6v": np.ascontiguousarray(x6v).astype(ml_dtypes.bfloat16),
            }
        )
    return in_maps


def assemble_output(results):
    """Stitch 8 per-core (2048, 64) outputs into (S, B, NE)."""
    out = np.zeros((S, B, NE), np.float32)
    for core in range(N_CORES):
        b, parity = core // 2, core % 2
        jos = JOS_BY_PARITY[parity]
        co = results[core]["out"]
        for t in range(NSLOT):
            out[jos[t] : jos[t] + JT, b, :] = co[t * JT : (t + 1) * JT, :]
    return out


def run_on_device(in_maps, trace=False):
    from concourse.bass_utils import run_bass_kernel_spmd

    if "nc" not in _NC_CACHE:
        _NC_CACHE["nc"] = build_graph()
    nc = _NC_CACHE["nc"]
    return run_bass_kernel_spmd(
        nc, in_maps, core_ids=list(range(N_CORES)), trace=trace
    )


def kernel(x, Wk, bk, Wq, bq, Wv, bv):
    in_maps = make_in_maps(x, Wk, bk, Wq, bq, Wv, bv)
    res = run_on_device(in_maps, trace=False)
    return assemble_output(res.results)
